# revision 22
# baseline (speedup 1.0000x reference)
"""Trainium2 Bass kernel for nn_BinaryLinear: out = sign(x @ sign(W).T + bias).

Strategy
--------
Data-parallel over the 8192-token dim: each of the 8 cores gets 1024 tokens
and the full weight matrix.

On-chip compute (per core) is the NT GEMM z.T = sign(W) @ x.T on the
TensorEngine with the contraction (in_features) on the partition dim:

  psum[outf, tok] = sum_k w_sgn_T[k, outf] * x_T[k, tok]

Precision/speed: a SINGLE float32r (TF32-like, truncated-mantissa) matmul
stream at 1.0 PE cycles/row — vs 1.5 cycles/row for an fp16-hi +
fp8-DoubleRow-lo split. x ships as plain fp32 (the PE rounds to fp32r);
sign(W) ships as e4m3 (+-1 exact; 16 MB vs 64 MB fp32) in block-major
layout and is widened to fp32r chunks by ScalarE Copy (~0.5 us per
[128,256] chunk) one block ahead of the matmuls. Measured flips vs the
fp32 reference: ~1090/33.5M, rel err ~1.1e-2, under the 2e-2 gate.

The PE stream runs ~227 ns per [128k x 128m x 512n] matmul; the kernel is
startup-bound on the 16 MB x stream. The FIRST output block is
double-width (512 outf, 8 PSUM groups, ~58 us of matmuls) so its compute
window covers the x arrival: x chunks split even/odd across the sync and
gpsimd DMA queues behind the two leading W-block stages, and every later
block finds all inputs resident. Converts for block mb+1 are emitted
before block mb's epilogue so the ScalarE FIFO never delays them behind
Sign ops. The last block runs group-major so the final Sign/store
overlaps its remaining matmuls.

The epilogue fuses bias-add + sign + PSUM->SBUF in one ScalarE activation
(bias is per-partition in the z.T layout); output is written as fp16 z.T
(+-1 exact) and untransposed/cast on the host.
"""

import numpy as np
import ml_dtypes

import concourse.tile as tile
import concourse.mybir as mybir
from concourse import bacc
from concourse.bass_utils import run_bass_kernel_spmd

N_CORES = 8
N_TOK = 8192
D_IN = 4096
D_OUT = 4096
P = 128
T = N_TOK // N_CORES  # 1024 tokens per core
KT = D_IN // P  # 32 contraction tiles
MT = D_OUT // P  # 32 out-feature tiles
M2 = 2  # m-tiles per W block (256 outf cols)
MB = MT // M2  # 16 W blocks
TB = 512  # token block (one PSUM bank of fp32)
NB = T // TB  # 2 token blocks per core
MW = M2 * P  # W block width (256)

F32 = mybir.dt.float32
F32R = mybir.dt.float32r
FP16 = mybir.dt.float16
FP8 = mybir.dt.float8e4
SIGN = mybir.ActivationFunctionType.Sign
COPY = mybir.ActivationFunctionType.Copy
E4M3 = ml_dtypes.float8_e4m3

_nc_cache = None


def build():
    """Build + compile the per-core Bass/Tile module (SPMD: same on all cores)."""
    global _nc_cache
    if _nc_cache is not None:
        return _nc_cache
    nc = bacc.Bacc("TRN2", target_bir_lowering=False, debug=False, num_devices=N_CORES)
    x_d = nc.dram_tensor("x_t", [D_IN, T], F32R, kind="ExternalInput").ap()
    w_d = nc.dram_tensor("w_blk8", [MB, P, KT * MW], FP8, kind="ExternalInput").ap()
    b_d = nc.dram_tensor("bias", [D_OUT], F32, kind="ExternalInput").ap()
    out_d = nc.dram_tensor("out_t", [D_OUT, T], FP16, kind="ExternalOutput").ap()

    with tile.TileContext(nc) as tc:
        with (
            tc.tile_pool(name="x", bufs=1) as x_pool,
            tc.tile_pool(name="wstage", bufs=3) as ws_pool,
            tc.tile_pool(name="wc01", bufs=12) as wc01_pool,
            tc.tile_pool(name="wcs", bufs=32) as wcs_pool,
            tc.tile_pool(name="bias", bufs=1) as b_pool,
            tc.tile_pool(name="out", bufs=6) as out_pool,
            tc.tile_pool(name="psum", bufs=8, space="PSUM") as psum_pool,
        ):
            def stage_w_block(mb, queue, split=1):
                # Stage a block-major [P, KT*256] e4m3 sign(W) block
                # (16 KB/partition lines); `split` sub-DMAs shorten the
                # first convert's dependency for the leading blocks.
                ws = ws_pool.tile([P, KT * MW], FP8, tag="ws", name=f"ws_{mb}")
                step = KT * MW // split
                for i in range(split):
                    queue.dma_start(
                        ws[:, i * step : (i + 1) * step],
                        w_d[mb, :, i * step : (i + 1) * step],
                    )
                return ws

            def convert_w_chunk(mb, ws, k):
                # Super-block converts: ScalarE activation Copy.
                wc = wc01_pool.tile([P, MW], F32R, tag="wc", name=f"wc_{mb}_{k}")
                nc.scalar.activation(wc[:], ws[:, k * MW : (k + 1) * MW], COPY)
                return wc

            def convert_w_chunk_dve(mb, ws, k):
                # Steady-block converts ride the idle DVE so they are not
                # FIFO-blocked behind the super-block's pool-paced converts.
                wc = wcs_pool.tile([P, MW], F32R, tag="wcs", name=f"wc_{mb}_{k}")
                nc.vector.tensor_copy(wc[:], ws[:, k * MW : (k + 1) * MW])
                return wc

            # x chunks 0,1 lead the two x queues so the first matmul's
            # moving operand lands as early as possible; W blocks 0 and 1
            # follow (split so the first converts start after one eighth of
            # a block).
            def x_chunk(k, queue):
                th = x_pool.tile([P, T], F32R, tag=f"x_{k}", name=f"x_{k}")
                queue.dma_start(th[:], x_d[k * P : (k + 1) * P, :])
                return th

            xt = [None] * KT
            xt[0] = x_chunk(0, nc.sync)
            xt[1] = x_chunk(1, nc.gpsimd)

            # PE warm-up: ~18 matmuls reading x chunk 0 into a scratch PSUM
            # bank (results discarded). They run as soon as x0 lands, while
            # the rest of the inputs stream in, so the HAM clock-gate is at
            # 8/8 (2.4 GHz) when the first real matmul issues — sized to end
            # right around the real stream's data-readiness.
            warm_ps = psum_pool.tile([P, TB], F32, tag="psum", name="warm_ps")
            for i in range(18):
                nc.tensor.matmul(
                    warm_ps[:], xt[0][:, 0:P], xt[0][:, 0:TB],
                    start=True, stop=True, skip_group_check=True,
                )

            # W blocks 0/1 quarters interleaved with the early x chunks on
            # each queue: the super-block needs x_k every ~1.8 us but W
            # quarters only ahead of the convert stream.
            ws0 = ws_pool.tile([P, KT * MW], FP8, tag="ws", name="ws_0")
            ws1 = ws_pool.tile([P, KT * MW], FP8, tag="ws", name="ws_1")
            QS = KT * MW // 8
            def w01_quarters(ws, wd_mb, queue, lo, hi):
                for i in range(lo, hi):
                    queue.dma_start(ws[:, i * QS : (i + 1) * QS],
                                    w_d[wd_mb, :, i * QS : (i + 1) * QS])
            w01_quarters(ws0, 0, nc.sync, 0, 2)
            w01_quarters(ws1, 1, nc.gpsimd, 0, 2)
            xt[2] = x_chunk(2, nc.sync)
            xt[3] = x_chunk(3, nc.gpsimd)
            w01_quarters(ws0, 0, nc.sync, 2, 4)
            w01_quarters(ws1, 1, nc.gpsimd, 2, 4)
            xt[4] = x_chunk(4, nc.sync)
            xt[5] = x_chunk(5, nc.gpsimd)
            w01_quarters(ws0, 0, nc.sync, 4, 8)
            w01_quarters(ws1, 1, nc.gpsimd, 4, 8)

            # Interleaved converts (0,k),(1,k) for the fused first block.
            wcs01 = [[], []]
            for k in range(KT):
                wcs01[0].append(convert_w_chunk(0, ws0, k))
                wcs01[1].append(convert_w_chunk(1, ws1, k))

            # Remaining x: even chunks on sync, odd on gpsimd (behind W0/W1).
            # W blocks 2.. are staged via cheap triggers on the scalar
            # queue (emitted after the leading converts) so the two x
            # streams are never displaced; outputs ride sync only after the
            # x stream has fully landed, bias rides gpsimd likewise.
            for k in range(6, KT):
                xt[k] = x_chunk(k, nc.sync if k % 2 == 0 else nc.gpsimd)
            wstage_cache = {
                2: stage_w_block(2, nc.scalar),
                3: stage_w_block(3, nc.scalar),
            }
            bias_sb = b_pool.tile([P, MT], F32, tag="bias")
            nc.gpsimd.dma_start(bias_sb[:], b_d.rearrange("(mo p) -> p mo", p=P))

            nsls = [slice(n * TB, (n + 1) * TB) for n in range(NB)]

            def epilogue(m, n, ps):
                osb = out_pool.tile([P, TB], FP16, tag="osb",
                                    name=f"osb_{m}_{n}")
                nc.scalar.activation(
                    osb[:], ps[:], SIGN, bias=bias_sb[:, m : m + 1]
                )
                nc.sync.dma_start(out_d[m * P : (m + 1) * P, nsls[n]], osb[:])

            # ---- Fused first super-block: W blocks 0+1, 512 outf, all 8
            # PSUM banks. Its ~58 us of matmuls cover the x stream arrival.
            psums0 = {
                (mi, n): psum_pool.tile([P, TB], F32, tag="psum",
                                        name=f"ps0_{n}_{mi}")
                for mi in range(2 * M2)
                for n in range(NB)
            }

            wc_cache = {}
            KS = KT - 4  # tail k-steps run group-major so groups drain early
            for k in range(KS):
                for mi in range(2 * M2):
                    b, half = divmod(mi, 2)
                    msl = slice(half * P, (half + 1) * P)
                    for n in range(NB):
                        nc.tensor.matmul(
                            psums0[(mi, n)][:],
                            wcs01[b][k][:, msl],
                            xt[k][:, nsls[n]],
                            start=(k == 0),
                            stop=False,
                        )
            ws2 = wstage_cache.pop(2)
            wc_cache[2] = [convert_w_chunk_dve(2, ws2, k) for k in range(KT)]
            # Tail: finish each PSUM group separately and drain it at once,
            # so block 2's bank anti-dependencies clear before it needs them.
            for mi in range(2 * M2):
                b, half = divmod(mi, 2)
                msl = slice(half * P, (half + 1) * P)
                for n in range(NB):
                    for k in range(KS, KT):
                        nc.tensor.matmul(
                            psums0[(mi, n)][:],
                            wcs01[b][k][:, msl],
                            xt[k][:, nsls[n]],
                            start=False,
                            stop=(k == KT - 1),
                        )
                    epilogue(mi, n, psums0[(mi, n)])

            # ---- Steady-state blocks 2..15.
            for mb in range(2, MB):
                wcs = wc_cache.pop(mb)
                pf = mb + 2
                if pf < MB:
                    wstage_cache[pf] = stage_w_block(pf, nc.scalar)
                psums = {
                    (mi, n): psum_pool.tile([P, TB], F32, tag="psum",
                                            name=f"ps_{mb}_{n}_{mi}")
                    for mi in range(M2)
                    for n in range(NB)
                }
                last = mb == MB - 1
                if not last:
                    for k in range(KT):
                        for mi in range(M2):
                            msl = slice(mi * P, (mi + 1) * P)
                            for n in range(NB):
                                nc.tensor.matmul(
                                    psums[(mi, n)][:],
                                    wcs[k][:, msl],
                                    xt[k][:, nsls[n]],
                                    start=(k == 0),
                                    stop=(k == KT - 1),
                                )
                    if mb + 1 < MB:
                        wsn = wstage_cache.pop(mb + 1)
                        wc_cache[mb + 1] = [
                            convert_w_chunk_dve(mb + 1, wsn, k)
                            for k in range(KT)
                        ]
                    for mi in range(M2):
                        for n in range(NB):
                            epilogue(mb * M2 + mi, n, psums[(mi, n)])
                else:
                    # Last block group-major: each PSUM group finishes early
                    # so its epilogue overlaps the remaining matmuls.
                    for mi in range(M2):
                        msl = slice(mi * P, (mi + 1) * P)
                        for n in range(NB):
                            for k in range(KT):
                                nc.tensor.matmul(
                                    psums[(mi, n)][:],
                                    wcs[k][:, msl],
                                    xt[k][:, nsls[n]],
                                    start=(k == 0),
                                    stop=(k == KT - 1),
                                )
                            epilogue(mb * M2 + mi, n, psums[(mi, n)])
    nc.compile()
    _nc_cache = nc
    return nc


def prep_in_maps(x, weight, bias):
    """Host-side layout prep: transpose x, e4m3 sign(W) blocks, token shards."""
    x = np.asarray(x, dtype=np.float32)
    weight = np.asarray(weight, dtype=np.float32)
    bias = np.asarray(bias, dtype=np.float32)

    x_t = np.ascontiguousarray(x.T)  # [D_IN, N_TOK] fp32

    # sign(W).T [D_IN, D_OUT] -> block-major [MB][P, KT*256]:
    # w_blk8[mb, p, k*256+j] = sign(W).T[k*128+p, mb*256+j]
    s_t = np.sign(weight).T.astype(E4M3)  # [D_IN, D_OUT]
    w_blk8 = np.ascontiguousarray(
        s_t.reshape(KT, P, MB, MW)  # [k, p, mb, j]
        .transpose(2, 1, 0, 3)      # [mb, p, k, j]
        .reshape(MB, P, KT * MW)
    )

    in_maps = []
    for c in range(N_CORES):
        sl = slice(c * T, (c + 1) * T)
        in_maps.append(
            {
                "x_t": np.ascontiguousarray(x_t[:, sl]),
                "w_blk8": w_blk8,
                "bias": bias,
            }
        )
    return in_maps


def run(x, weight, bias, **spmd_kwargs):
    """Run on the 8 cores; returns (full_output, BassKernelResults)."""
    nc = build()
    in_maps = prep_in_maps(x, weight, bias)
    res = run_bass_kernel_spmd(nc, in_maps, core_ids=list(range(N_CORES)), **spmd_kwargs)
    out = np.empty((N_TOK, D_OUT), dtype=np.float32)
    for c in range(N_CORES):
        out[c * T : (c + 1) * T, :] = res.results[c]["out_t"].T.astype(np.float32)
    return out, res


def kernel(x, weight, bias):
    out, _ = run(x, weight, bias)
    return out


# revision 23
# speedup vs baseline: 1.0009x; 1.0009x over previous
"""Trainium2 Bass kernel for nn_BinaryLinear: out = sign(x @ sign(W).T + bias).

Strategy
--------
Data-parallel over the 8192-token dim: each of the 8 cores gets 1024 tokens
and the full weight matrix.

On-chip compute (per core) is the NT GEMM z.T = sign(W) @ x.T on the
TensorEngine with the contraction (in_features) on the partition dim:

  psum[outf, tok] = sum_k w_sgn_T[k, outf] * x_T[k, tok]

Precision/speed: a SINGLE float32r (TF32-like, truncated-mantissa) matmul
stream at 1.0 PE cycles/row — vs 1.5 cycles/row for an fp16-hi +
fp8-DoubleRow-lo split. x ships as plain fp32 (the PE rounds to fp32r);
sign(W) ships as e4m3 (+-1 exact; 16 MB vs 64 MB fp32) in block-major
layout and is widened to fp32r chunks by ScalarE Copy (~0.5 us per
[128,256] chunk) one block ahead of the matmuls. Measured flips vs the
fp32 reference: ~1090/33.5M, rel err ~1.1e-2, under the 2e-2 gate.

The PE stream runs ~227 ns per [128k x 128m x 512n] matmul; the kernel is
startup-bound on the 16 MB x stream. The FIRST output block is
double-width (512 outf, 8 PSUM groups, ~58 us of matmuls) so its compute
window covers the x arrival: x chunks split even/odd across the sync and
gpsimd DMA queues behind the two leading W-block stages, and every later
block finds all inputs resident. Converts for block mb+1 are emitted
before block mb's epilogue so the ScalarE FIFO never delays them behind
Sign ops. The last block runs group-major so the final Sign/store
overlaps its remaining matmuls.

The epilogue fuses bias-add + sign + PSUM->SBUF in one ScalarE activation
(bias is per-partition in the z.T layout); output is written as fp16 z.T
(+-1 exact) and untransposed/cast on the host.
"""

import numpy as np
import ml_dtypes

import concourse.tile as tile
import concourse.mybir as mybir
from concourse import bacc
from concourse.bass_utils import run_bass_kernel_spmd

N_CORES = 8
N_TOK = 8192
D_IN = 4096
D_OUT = 4096
P = 128
T = N_TOK // N_CORES  # 1024 tokens per core
KT = D_IN // P  # 32 contraction tiles
MT = D_OUT // P  # 32 out-feature tiles
M2 = 2  # m-tiles per W block (256 outf cols)
MB = MT // M2  # 16 W blocks
TB = 512  # token block (one PSUM bank of fp32)
NB = T // TB  # 2 token blocks per core
MW = M2 * P  # W block width (256)

F32 = mybir.dt.float32
F32R = mybir.dt.float32r
FP16 = mybir.dt.float16
FP8 = mybir.dt.float8e4
SIGN = mybir.ActivationFunctionType.Sign
COPY = mybir.ActivationFunctionType.Copy
E4M3 = ml_dtypes.float8_e4m3

_nc_cache = None


def build():
    """Build + compile the per-core Bass/Tile module (SPMD: same on all cores)."""
    global _nc_cache
    if _nc_cache is not None:
        return _nc_cache
    nc = bacc.Bacc("TRN2", target_bir_lowering=False, debug=False, num_devices=N_CORES)
    x_d = nc.dram_tensor("x_t", [D_IN, T], F32R, kind="ExternalInput").ap()
    w_d = nc.dram_tensor("w_blk8", [MB, P, KT * MW], FP8, kind="ExternalInput").ap()
    b_d = nc.dram_tensor("bias", [D_OUT], F32, kind="ExternalInput").ap()
    out_d = nc.dram_tensor("out_t", [D_OUT, T], FP16, kind="ExternalOutput").ap()

    with tile.TileContext(nc) as tc:
        with (
            tc.tile_pool(name="x", bufs=1) as x_pool,
            tc.tile_pool(name="wstage", bufs=4) as ws_pool,
            tc.tile_pool(name="wc01", bufs=10) as wc01_pool,
            tc.tile_pool(name="wcs", bufs=28) as wcs_pool,
            tc.tile_pool(name="bias", bufs=1) as b_pool,
            tc.tile_pool(name="out", bufs=6) as out_pool,
            tc.tile_pool(name="psum", bufs=8, space="PSUM") as psum_pool,
        ):
            def stage_w_block(mb, queue, split=1):
                # Stage a block-major [P, KT*256] e4m3 sign(W) block
                # (16 KB/partition lines); `split` sub-DMAs shorten the
                # first convert's dependency for the leading blocks.
                ws = ws_pool.tile([P, KT * MW], FP8, tag="ws", name=f"ws_{mb}")
                step = KT * MW // split
                for i in range(split):
                    queue.dma_start(
                        ws[:, i * step : (i + 1) * step],
                        w_d[mb, :, i * step : (i + 1) * step],
                    )
                return ws

            def convert_w_chunk(mb, ws, k):
                # Super-block converts: ScalarE activation Copy.
                wc = wc01_pool.tile([P, MW], F32R, tag="wc", name=f"wc_{mb}_{k}")
                nc.scalar.activation(wc[:], ws[:, k * MW : (k + 1) * MW], COPY)
                return wc

            def convert_w_chunk_dve(mb, ws, k):
                # Steady-block converts ride the idle DVE so they are not
                # FIFO-blocked behind the super-block's pool-paced converts.
                wc = wcs_pool.tile([P, MW], F32R, tag="wcs", name=f"wc_{mb}_{k}")
                nc.vector.tensor_copy(wc[:], ws[:, k * MW : (k + 1) * MW])
                return wc

            # x chunks 0,1 lead the two x queues so the first matmul's
            # moving operand lands as early as possible; W blocks 0 and 1
            # follow (split so the first converts start after one eighth of
            # a block).
            def x_chunk(k, queue):
                th = x_pool.tile([P, T], F32R, tag=f"x_{k}", name=f"x_{k}")
                queue.dma_start(th[:], x_d[k * P : (k + 1) * P, :])
                return th

            xt = [None] * KT
            xt[0] = x_chunk(0, nc.sync)
            xt[1] = x_chunk(1, nc.gpsimd)

            # PE warm-up: ~18 matmuls reading x chunk 0 into a scratch PSUM
            # bank (results discarded). They run as soon as x0 lands, while
            # the rest of the inputs stream in, so the HAM clock-gate is at
            # 8/8 (2.4 GHz) when the first real matmul issues — sized to end
            # right around the real stream's data-readiness.
            warm_ps = psum_pool.tile([P, TB], F32, tag="psum", name="warm_ps")
            for i in range(18):
                nc.tensor.matmul(
                    warm_ps[:], xt[0][:, 0:P], xt[0][:, 0:TB],
                    start=True, stop=True, skip_group_check=True,
                )

            # W blocks 0/1 quarters interleaved with the early x chunks on
            # each queue: the super-block needs x_k every ~1.8 us but W
            # quarters only ahead of the convert stream.
            ws0 = ws_pool.tile([P, KT * MW], FP8, tag="ws", name="ws_0")
            ws1 = ws_pool.tile([P, KT * MW], FP8, tag="ws", name="ws_1")
            QS = KT * MW // 8
            def w01_quarters(ws, wd_mb, queue, lo, hi):
                for i in range(lo, hi):
                    queue.dma_start(ws[:, i * QS : (i + 1) * QS],
                                    w_d[wd_mb, :, i * QS : (i + 1) * QS])
            w01_quarters(ws0, 0, nc.sync, 0, 2)
            w01_quarters(ws1, 1, nc.gpsimd, 0, 2)
            xt[2] = x_chunk(2, nc.sync)
            xt[3] = x_chunk(3, nc.gpsimd)
            w01_quarters(ws0, 0, nc.sync, 2, 4)
            w01_quarters(ws1, 1, nc.gpsimd, 2, 4)
            xt[4] = x_chunk(4, nc.sync)
            xt[5] = x_chunk(5, nc.gpsimd)
            w01_quarters(ws0, 0, nc.sync, 4, 8)
            w01_quarters(ws1, 1, nc.gpsimd, 4, 8)

            # Interleaved converts (0,k),(1,k) for the fused first block.
            wcs01 = [[], []]
            for k in range(KT):
                wcs01[0].append(convert_w_chunk(0, ws0, k))
                wcs01[1].append(convert_w_chunk(1, ws1, k))

            # Remaining x: even chunks on sync, odd on gpsimd (behind W0/W1).
            # W blocks 2.. are staged via cheap triggers on the scalar
            # queue (emitted after the leading converts) so the two x
            # streams are never displaced; outputs ride sync only after the
            # x stream has fully landed, bias rides gpsimd likewise.
            for k in range(6, KT):
                xt[k] = x_chunk(k, nc.sync if k % 2 == 0 else nc.gpsimd)
            wstage_cache = {
                2: stage_w_block(2, nc.scalar),
                3: stage_w_block(3, nc.scalar),
            }
            bias_sb = b_pool.tile([P, MT], F32, tag="bias")
            nc.gpsimd.dma_start(bias_sb[:], b_d.rearrange("(mo p) -> p mo", p=P))

            nsls = [slice(n * TB, (n + 1) * TB) for n in range(NB)]

            def epilogue(m, n, ps):
                osb = out_pool.tile([P, TB], FP16, tag="osb",
                                    name=f"osb_{m}_{n}")
                nc.scalar.activation(
                    osb[:], ps[:], SIGN, bias=bias_sb[:, m : m + 1]
                )
                nc.sync.dma_start(out_d[m * P : (m + 1) * P, nsls[n]], osb[:])

            # ---- Fused first super-block: W blocks 0+1, 512 outf, all 8
            # PSUM banks. Its ~58 us of matmuls cover the x stream arrival.
            psums0 = {
                (mi, n): psum_pool.tile([P, TB], F32, tag="psum",
                                        name=f"ps0_{n}_{mi}")
                for mi in range(2 * M2)
                for n in range(NB)
            }

            wc_cache = {}
            KS = KT - 4  # tail k-steps run group-major so groups drain early
            for k in range(KS):
                for mi in range(2 * M2):
                    b, half = divmod(mi, 2)
                    msl = slice(half * P, (half + 1) * P)
                    for n in range(NB):
                        nc.tensor.matmul(
                            psums0[(mi, n)][:],
                            wcs01[b][k][:, msl],
                            xt[k][:, nsls[n]],
                            start=(k == 0),
                            stop=False,
                        )
            ws2 = wstage_cache.pop(2)
            wc_cache[2] = [convert_w_chunk_dve(2, ws2, k) for k in range(KT)]
            # Tail: finish each PSUM group separately and drain it at once,
            # so block 2's bank anti-dependencies clear before it needs them.
            for mi in range(2 * M2):
                b, half = divmod(mi, 2)
                msl = slice(half * P, (half + 1) * P)
                for n in range(NB):
                    for k in range(KS, KT):
                        nc.tensor.matmul(
                            psums0[(mi, n)][:],
                            wcs01[b][k][:, msl],
                            xt[k][:, nsls[n]],
                            start=False,
                            stop=(k == KT - 1),
                        )
                    epilogue(mi, n, psums0[(mi, n)])

            # ---- Steady-state blocks 2..15.
            for mb in range(2, MB):
                wcs = wc_cache.pop(mb)
                pf = mb + 2
                if pf < MB:
                    wstage_cache[pf] = stage_w_block(pf, nc.scalar)
                psums = {
                    (mi, n): psum_pool.tile([P, TB], F32, tag="psum",
                                            name=f"ps_{mb}_{n}_{mi}")
                    for mi in range(M2)
                    for n in range(NB)
                }
                last = mb == MB - 1
                if not last:
                    for k in range(KT):
                        for mi in range(M2):
                            msl = slice(mi * P, (mi + 1) * P)
                            for n in range(NB):
                                nc.tensor.matmul(
                                    psums[(mi, n)][:],
                                    wcs[k][:, msl],
                                    xt[k][:, nsls[n]],
                                    start=(k == 0),
                                    stop=(k == KT - 1),
                                )
                    if mb + 1 < MB:
                        wsn = wstage_cache.pop(mb + 1)
                        wc_cache[mb + 1] = [
                            convert_w_chunk_dve(mb + 1, wsn, k)
                            for k in range(KT)
                        ]
                    for mi in range(M2):
                        for n in range(NB):
                            epilogue(mb * M2 + mi, n, psums[(mi, n)])
                else:
                    # Last block group-major: each PSUM group finishes early
                    # so its epilogue overlaps the remaining matmuls.
                    for mi in range(M2):
                        msl = slice(mi * P, (mi + 1) * P)
                        for n in range(NB):
                            for k in range(KT):
                                nc.tensor.matmul(
                                    psums[(mi, n)][:],
                                    wcs[k][:, msl],
                                    xt[k][:, nsls[n]],
                                    start=(k == 0),
                                    stop=(k == KT - 1),
                                )
                            epilogue(mb * M2 + mi, n, psums[(mi, n)])
    nc.compile()
    _nc_cache = nc
    return nc


def prep_in_maps(x, weight, bias):
    """Host-side layout prep: transpose x, e4m3 sign(W) blocks, token shards."""
    x = np.asarray(x, dtype=np.float32)
    weight = np.asarray(weight, dtype=np.float32)
    bias = np.asarray(bias, dtype=np.float32)

    x_t = np.ascontiguousarray(x.T)  # [D_IN, N_TOK] fp32

    # sign(W).T [D_IN, D_OUT] -> block-major [MB][P, KT*256]:
    # w_blk8[mb, p, k*256+j] = sign(W).T[k*128+p, mb*256+j]
    s_t = np.sign(weight).T.astype(E4M3)  # [D_IN, D_OUT]
    w_blk8 = np.ascontiguousarray(
        s_t.reshape(KT, P, MB, MW)  # [k, p, mb, j]
        .transpose(2, 1, 0, 3)      # [mb, p, k, j]
        .reshape(MB, P, KT * MW)
    )

    in_maps = []
    for c in range(N_CORES):
        sl = slice(c * T, (c + 1) * T)
        in_maps.append(
            {
                "x_t": np.ascontiguousarray(x_t[:, sl]),
                "w_blk8": w_blk8,
                "bias": bias,
            }
        )
    return in_maps


def run(x, weight, bias, **spmd_kwargs):
    """Run on the 8 cores; returns (full_output, BassKernelResults)."""
    nc = build()
    in_maps = prep_in_maps(x, weight, bias)
    res = run_bass_kernel_spmd(nc, in_maps, core_ids=list(range(N_CORES)), **spmd_kwargs)
    out = np.empty((N_TOK, D_OUT), dtype=np.float32)
    for c in range(N_CORES):
        out[c * T : (c + 1) * T, :] = res.results[c]["out_t"].T.astype(np.float32)
    return out, res


def kernel(x, weight, bias):
    out, _ = run(x, weight, bias)
    return out


# revision 24
# speedup vs baseline: 1.0149x; 1.0139x over previous
"""Trainium2 Bass kernel for nn_BinaryLinear: out = sign(x @ sign(W).T + bias).

Strategy
--------
Data-parallel over the 8192-token dim: each of the 8 cores gets 1024 tokens
and the full weight matrix.

On-chip compute (per core) is the NT GEMM z.T = sign(W) @ x.T on the
TensorEngine with the contraction (in_features) on the partition dim:

  psum[outf, tok] = sum_k w_sgn_T[k, outf] * x_T[k, tok]

Precision/speed: a SINGLE float32r (TF32-like, truncated-mantissa) matmul
stream at 1.0 PE cycles/row — vs 1.5 cycles/row for an fp16-hi +
fp8-DoubleRow-lo split. x ships as plain fp32 (the PE rounds to fp32r);
sign(W) ships as e4m3 (+-1 exact; 16 MB vs 64 MB fp32) in block-major
layout and is widened to fp32r chunks by ScalarE Copy (~0.5 us per
[128,256] chunk) one block ahead of the matmuls. Measured flips vs the
fp32 reference: ~1090/33.5M, rel err ~1.1e-2, under the 2e-2 gate.

The PE stream runs ~227 ns per [128k x 128m x 512n] matmul; the kernel is
startup-bound on the 16 MB x stream. The FIRST output block is
double-width (512 outf, 8 PSUM groups, ~58 us of matmuls) so its compute
window covers the x arrival: x chunks split even/odd across the sync and
gpsimd DMA queues behind the two leading W-block stages, and every later
block finds all inputs resident. Converts for block mb+1 are emitted
before block mb's epilogue so the ScalarE FIFO never delays them behind
Sign ops. The last block runs group-major so the final Sign/store
overlaps its remaining matmuls.

The epilogue fuses bias-add + sign + PSUM->SBUF in one ScalarE activation
(bias is per-partition in the z.T layout); output is written as fp16 z.T
(+-1 exact) and untransposed/cast on the host.
"""

import numpy as np
import ml_dtypes

import concourse.tile as tile
import concourse.mybir as mybir
from concourse import bacc
from concourse.bass_utils import run_bass_kernel_spmd

N_CORES = 8
N_TOK = 8192
D_IN = 4096
D_OUT = 4096
P = 128
T = N_TOK // N_CORES  # 1024 tokens per core
KT = D_IN // P  # 32 contraction tiles
MT = D_OUT // P  # 32 out-feature tiles
M2 = 2  # m-tiles per W block (256 outf cols)
MB = MT // M2  # 16 W blocks
TB = 512  # token block (one PSUM bank of fp32)
NB = T // TB  # 2 token blocks per core
MW = M2 * P  # W block width (256)

F32 = mybir.dt.float32
F32R = mybir.dt.float32r
FP16 = mybir.dt.float16
FP8 = mybir.dt.float8e4
SIGN = mybir.ActivationFunctionType.Sign
COPY = mybir.ActivationFunctionType.Copy
E4M3 = ml_dtypes.float8_e4m3

_nc_cache = None


def build():
    """Build + compile the per-core Bass/Tile module (SPMD: same on all cores)."""
    global _nc_cache
    if _nc_cache is not None:
        return _nc_cache
    nc = bacc.Bacc("TRN2", target_bir_lowering=False, debug=False, num_devices=N_CORES)
    x_d = nc.dram_tensor("x_t", [D_IN, T], F32R, kind="ExternalInput").ap()
    w_d = nc.dram_tensor("w_blk8", [MB, P, KT * MW], FP8, kind="ExternalInput").ap()
    b_d = nc.dram_tensor("bias", [D_OUT], F32, kind="ExternalInput").ap()
    out_d = nc.dram_tensor("out_t", [D_OUT, T], FP16, kind="ExternalOutput").ap()

    with tile.TileContext(nc) as tc:
        with (
            tc.tile_pool(name="x", bufs=1) as x_pool,
            tc.tile_pool(name="wstage", bufs=4) as ws_pool,
            tc.tile_pool(name="wc01", bufs=10) as wc01_pool,
            tc.tile_pool(name="wcs", bufs=28) as wcs_pool,
            tc.tile_pool(name="bias", bufs=1) as b_pool,
            tc.tile_pool(name="out", bufs=6) as out_pool,
            tc.tile_pool(name="psum", bufs=8, space="PSUM") as psum_pool,
        ):
            def stage_w_block(mb, queue, split=1):
                # Stage a block-major [P, KT*256] e4m3 sign(W) block
                # (16 KB/partition lines); `split` sub-DMAs shorten the
                # first convert's dependency for the leading blocks.
                ws = ws_pool.tile([P, KT * MW], FP8, tag="ws", name=f"ws_{mb}")
                step = KT * MW // split
                for i in range(split):
                    queue.dma_start(
                        ws[:, i * step : (i + 1) * step],
                        w_d[mb, :, i * step : (i + 1) * step],
                    )
                return ws

            def convert_w_chunk(mb, ws, k):
                # Super-block converts: ScalarE activation Copy.
                wc = wc01_pool.tile([P, MW], F32R, tag="wc", name=f"wc_{mb}_{k}")
                nc.scalar.activation(wc[:], ws[:, k * MW : (k + 1) * MW], COPY)
                return wc

            def convert_w_chunk_dve(mb, ws, k):
                # Steady-block converts ride the idle DVE so they are not
                # FIFO-blocked behind the super-block's pool-paced converts.
                wc = wcs_pool.tile([P, MW], F32R, tag="wcs", name=f"wc_{mb}_{k}")
                nc.vector.tensor_copy(wc[:], ws[:, k * MW : (k + 1) * MW])
                return wc

            # x chunks 0,1 lead the two x queues so the first matmul's
            # moving operand lands as early as possible; W blocks 0 and 1
            # follow (split so the first converts start after one eighth of
            # a block).
            def x_chunk(k, queue):
                # Two half-DMAs (n0 then n1): the k-batch's n=0 matmuls and
                # the warm-up gate on half the bytes, halving arrival latency.
                th = x_pool.tile([P, T], F32R, tag=f"x_{k}", name=f"x_{k}")
                queue.dma_start(th[:, 0:TB], x_d[k * P : (k + 1) * P, 0:TB])
                queue.dma_start(th[:, TB:T], x_d[k * P : (k + 1) * P, TB:T])
                return th

            xt = [None] * KT
            xt[0] = x_chunk(0, nc.sync)
            xt[1] = x_chunk(1, nc.gpsimd)

            # PE warm-up: ~18 matmuls reading x chunk 0 into a scratch PSUM
            # bank (results discarded). They run as soon as x0 lands, while
            # the rest of the inputs stream in, so the HAM clock-gate is at
            # 8/8 (2.4 GHz) when the first real matmul issues — sized to end
            # right around the real stream's data-readiness.
            warm_ps = psum_pool.tile([P, TB], F32, tag="psum", name="warm_ps")
            for i in range(30):
                nc.tensor.matmul(
                    warm_ps[:], xt[0][:, 0:P], xt[0][:, 0:TB],
                    start=True, stop=True, skip_group_check=True,
                )

            # W blocks 0/1 quarters interleaved with the early x chunks on
            # each queue: the super-block needs x_k every ~1.8 us but W
            # quarters only ahead of the convert stream.
            ws0 = ws_pool.tile([P, KT * MW], FP8, tag="ws", name="ws_0")
            ws1 = ws_pool.tile([P, KT * MW], FP8, tag="ws", name="ws_1")
            QS = KT * MW // 8
            def w01_quarters(ws, wd_mb, queue, lo, hi):
                for i in range(lo, hi):
                    queue.dma_start(ws[:, i * QS : (i + 1) * QS],
                                    w_d[wd_mb, :, i * QS : (i + 1) * QS])
            w01_quarters(ws0, 0, nc.sync, 0, 2)
            w01_quarters(ws1, 1, nc.gpsimd, 0, 2)
            xt[2] = x_chunk(2, nc.sync)
            xt[3] = x_chunk(3, nc.gpsimd)
            w01_quarters(ws0, 0, nc.sync, 2, 4)
            w01_quarters(ws1, 1, nc.gpsimd, 2, 4)
            xt[4] = x_chunk(4, nc.sync)
            xt[5] = x_chunk(5, nc.gpsimd)
            w01_quarters(ws0, 0, nc.sync, 4, 8)
            w01_quarters(ws1, 1, nc.gpsimd, 4, 8)

            # Interleaved converts (0,k),(1,k) for the fused first block.
            wcs01 = [[], []]
            for k in range(KT):
                wcs01[0].append(convert_w_chunk(0, ws0, k))
                wcs01[1].append(convert_w_chunk(1, ws1, k))

            # Remaining x: even chunks on sync, odd on gpsimd (behind W0/W1).
            # W blocks 2.. are staged via cheap triggers on the scalar
            # queue (emitted after the leading converts) so the two x
            # streams are never displaced; outputs ride sync only after the
            # x stream has fully landed, bias rides gpsimd likewise.
            for k in range(6, KT):
                xt[k] = x_chunk(k, nc.sync if k % 2 == 0 else nc.gpsimd)
            wstage_cache = {
                2: stage_w_block(2, nc.scalar),
                3: stage_w_block(3, nc.scalar),
            }
            bias_sb = b_pool.tile([P, MT], F32, tag="bias")
            nc.gpsimd.dma_start(bias_sb[:], b_d.rearrange("(mo p) -> p mo", p=P))

            nsls = [slice(n * TB, (n + 1) * TB) for n in range(NB)]

            def epilogue(m, n, ps):
                osb = out_pool.tile([P, TB], FP16, tag="osb",
                                    name=f"osb_{m}_{n}")
                nc.scalar.activation(
                    osb[:], ps[:], SIGN, bias=bias_sb[:, m : m + 1]
                )
                nc.sync.dma_start(out_d[m * P : (m + 1) * P, nsls[n]], osb[:])

            # ---- Fused first super-block: W blocks 0+1, 512 outf, all 8
            # PSUM banks. Its ~58 us of matmuls cover the x stream arrival.
            psums0 = {
                (mi, n): psum_pool.tile([P, TB], F32, tag="psum",
                                        name=f"ps0_{n}_{mi}")
                for mi in range(2 * M2)
                for n in range(NB)
            }

            wc_cache = {}
            KS = KT - 4  # tail k-steps run group-major so groups drain early
            for k in range(KS):
                for mi in range(2 * M2):
                    b, half = divmod(mi, 2)
                    msl = slice(half * P, (half + 1) * P)
                    for n in range(NB):
                        nc.tensor.matmul(
                            psums0[(mi, n)][:],
                            wcs01[b][k][:, msl],
                            xt[k][:, nsls[n]],
                            start=(k == 0),
                            stop=False,
                        )
            ws2 = wstage_cache.pop(2)
            wc_cache[2] = [convert_w_chunk_dve(2, ws2, k) for k in range(KT)]
            # Tail: finish each PSUM group separately and drain it at once,
            # so block 2's bank anti-dependencies clear before it needs them.
            for mi in range(2 * M2):
                b, half = divmod(mi, 2)
                msl = slice(half * P, (half + 1) * P)
                for n in range(NB):
                    for k in range(KS, KT):
                        nc.tensor.matmul(
                            psums0[(mi, n)][:],
                            wcs01[b][k][:, msl],
                            xt[k][:, nsls[n]],
                            start=False,
                            stop=(k == KT - 1),
                        )
                    epilogue(mi, n, psums0[(mi, n)])

            # ---- Steady-state blocks 2..15.
            for mb in range(2, MB):
                wcs = wc_cache.pop(mb)
                pf = mb + 2
                if pf < MB:
                    wstage_cache[pf] = stage_w_block(pf, nc.scalar)
                psums = {
                    (mi, n): psum_pool.tile([P, TB], F32, tag="psum",
                                            name=f"ps_{mb}_{n}_{mi}")
                    for mi in range(M2)
                    for n in range(NB)
                }
                last = mb == MB - 1
                if not last:
                    for k in range(KT):
                        for mi in range(M2):
                            msl = slice(mi * P, (mi + 1) * P)
                            for n in range(NB):
                                nc.tensor.matmul(
                                    psums[(mi, n)][:],
                                    wcs[k][:, msl],
                                    xt[k][:, nsls[n]],
                                    start=(k == 0),
                                    stop=(k == KT - 1),
                                )
                    if mb + 1 < MB:
                        wsn = wstage_cache.pop(mb + 1)
                        wc_cache[mb + 1] = [
                            convert_w_chunk_dve(mb + 1, wsn, k)
                            for k in range(KT)
                        ]
                    for mi in range(M2):
                        for n in range(NB):
                            epilogue(mb * M2 + mi, n, psums[(mi, n)])
                else:
                    # Last block group-major: each PSUM group finishes early
                    # so its epilogue overlaps the remaining matmuls.
                    for mi in range(M2):
                        msl = slice(mi * P, (mi + 1) * P)
                        for n in range(NB):
                            for k in range(KT):
                                nc.tensor.matmul(
                                    psums[(mi, n)][:],
                                    wcs[k][:, msl],
                                    xt[k][:, nsls[n]],
                                    start=(k == 0),
                                    stop=(k == KT - 1),
                                )
                            epilogue(mb * M2 + mi, n, psums[(mi, n)])
    nc.compile()
    _nc_cache = nc
    return nc


def prep_in_maps(x, weight, bias):
    """Host-side layout prep: transpose x, e4m3 sign(W) blocks, token shards."""
    x = np.asarray(x, dtype=np.float32)
    weight = np.asarray(weight, dtype=np.float32)
    bias = np.asarray(bias, dtype=np.float32)

    x_t = np.ascontiguousarray(x.T)  # [D_IN, N_TOK] fp32

    # sign(W).T [D_IN, D_OUT] -> block-major [MB][P, KT*256]:
    # w_blk8[mb, p, k*256+j] = sign(W).T[k*128+p, mb*256+j]
    s_t = np.sign(weight).T.astype(E4M3)  # [D_IN, D_OUT]
    w_blk8 = np.ascontiguousarray(
        s_t.reshape(KT, P, MB, MW)  # [k, p, mb, j]
        .transpose(2, 1, 0, 3)      # [mb, p, k, j]
        .reshape(MB, P, KT * MW)
    )

    in_maps = []
    for c in range(N_CORES):
        sl = slice(c * T, (c + 1) * T)
        in_maps.append(
            {
                "x_t": np.ascontiguousarray(x_t[:, sl]),
                "w_blk8": w_blk8,
                "bias": bias,
            }
        )
    return in_maps


def run(x, weight, bias, **spmd_kwargs):
    """Run on the 8 cores; returns (full_output, BassKernelResults)."""
    nc = build()
    in_maps = prep_in_maps(x, weight, bias)
    res = run_bass_kernel_spmd(nc, in_maps, core_ids=list(range(N_CORES)), **spmd_kwargs)
    out = np.empty((N_TOK, D_OUT), dtype=np.float32)
    for c in range(N_CORES):
        out[c * T : (c + 1) * T, :] = res.results[c]["out_t"].T.astype(np.float32)
    return out, res


def kernel(x, weight, bias):
    out, _ = run(x, weight, bias)
    return out


# revision 26
# speedup vs baseline: 1.0176x; 1.0027x over previous
"""Trainium2 Bass kernel for nn_BinaryLinear: out = sign(x @ sign(W).T + bias).

Strategy
--------
Data-parallel over the 8192-token dim: each of the 8 cores gets 1024 tokens
and the full weight matrix.

On-chip compute (per core) is the NT GEMM z.T = sign(W) @ x.T on the
TensorEngine with the contraction (in_features) on the partition dim:

  psum[outf, tok] = sum_k w_sgn_T[k, outf] * x_T[k, tok]

Precision/speed: a SINGLE float32r (TF32-like, truncated-mantissa) matmul
stream at 1.0 PE cycles/row — vs 1.5 cycles/row for an fp16-hi +
fp8-DoubleRow-lo split. x ships as plain fp32 (the PE rounds to fp32r);
sign(W) ships as e4m3 (+-1 exact; 16 MB vs 64 MB fp32) in block-major
layout and is widened to fp32r chunks by ScalarE Copy (~0.5 us per
[128,256] chunk) one block ahead of the matmuls. Measured flips vs the
fp32 reference: ~1090/33.5M, rel err ~1.1e-2, under the 2e-2 gate.

The PE stream runs ~227 ns per [128k x 128m x 512n] matmul; the kernel is
startup-bound on the 16 MB x stream. The FIRST output block is
double-width (512 outf, 8 PSUM groups, ~58 us of matmuls) so its compute
window covers the x arrival: x chunks split even/odd across the sync and
gpsimd DMA queues behind the two leading W-block stages, and every later
block finds all inputs resident. Converts for block mb+1 are emitted
before block mb's epilogue so the ScalarE FIFO never delays them behind
Sign ops. The last block runs group-major so the final Sign/store
overlaps its remaining matmuls.

The epilogue fuses bias-add + sign + PSUM->SBUF in one ScalarE activation
(bias is per-partition in the z.T layout); output is written as fp16 z.T
(+-1 exact) and untransposed/cast on the host.
"""

import numpy as np
import ml_dtypes

import concourse.tile as tile
import concourse.mybir as mybir
from concourse import bacc
from concourse.bass_utils import run_bass_kernel_spmd

N_CORES = 8
N_TOK = 8192
D_IN = 4096
D_OUT = 4096
P = 128
T = N_TOK // N_CORES  # 1024 tokens per core
KT = D_IN // P  # 32 contraction tiles
MT = D_OUT // P  # 32 out-feature tiles
M2 = 2  # m-tiles per W block (256 outf cols)
MB = MT // M2  # 16 W blocks
TB = 512  # token block (one PSUM bank of fp32)
NB = T // TB  # 2 token blocks per core
MW = M2 * P  # W block width (256)

F32 = mybir.dt.float32
F32R = mybir.dt.float32r
FP16 = mybir.dt.float16
FP8 = mybir.dt.float8e4
SIGN = mybir.ActivationFunctionType.Sign
COPY = mybir.ActivationFunctionType.Copy
E4M3 = ml_dtypes.float8_e4m3

_nc_cache = None


def build():
    """Build + compile the per-core Bass/Tile module (SPMD: same on all cores)."""
    global _nc_cache
    if _nc_cache is not None:
        return _nc_cache
    nc = bacc.Bacc("TRN2", target_bir_lowering=False, debug=False, num_devices=N_CORES)
    x_d = nc.dram_tensor("x_t", [D_IN, T], F32R, kind="ExternalInput").ap()
    w_d = nc.dram_tensor("w_blk8", [MB, P, KT * MW], FP8, kind="ExternalInput").ap()
    b_d = nc.dram_tensor("bias", [D_OUT], F32, kind="ExternalInput").ap()
    out_d = nc.dram_tensor("out_t", [D_OUT, T], FP16, kind="ExternalOutput").ap()

    with tile.TileContext(nc) as tc:
        with (
            tc.tile_pool(name="x", bufs=1) as x_pool,
            tc.tile_pool(name="wstage", bufs=4) as ws_pool,
            tc.tile_pool(name="wc01", bufs=10) as wc01_pool,
            tc.tile_pool(name="wcs", bufs=28) as wcs_pool,
            tc.tile_pool(name="bias", bufs=1) as b_pool,
            tc.tile_pool(name="out", bufs=6) as out_pool,
            tc.tile_pool(name="psum", bufs=8, space="PSUM") as psum_pool,
        ):
            def stage_w_block(mb, queue, split=1):
                # Stage a block-major [P, KT*256] e4m3 sign(W) block
                # (16 KB/partition lines); `split` sub-DMAs shorten the
                # first convert's dependency for the leading blocks.
                ws = ws_pool.tile([P, KT * MW], FP8, tag="ws", name=f"ws_{mb}")
                step = KT * MW // split
                for i in range(split):
                    queue.dma_start(
                        ws[:, i * step : (i + 1) * step],
                        w_d[mb, :, i * step : (i + 1) * step],
                    )
                return ws

            def convert_w_chunk(mb, ws, k):
                # Super-block converts: ScalarE activation Copy.
                wc = wc01_pool.tile([P, MW], F32R, tag="wc", name=f"wc_{mb}_{k}")
                nc.scalar.activation(wc[:], ws[:, k * MW : (k + 1) * MW], COPY)
                return wc

            def convert_w_chunk_dve(mb, ws, k):
                # Steady-block converts ride the idle DVE so they are not
                # FIFO-blocked behind the super-block's pool-paced converts.
                wc = wcs_pool.tile([P, MW], F32R, tag="wcs", name=f"wc_{mb}_{k}")
                nc.vector.tensor_copy(wc[:], ws[:, k * MW : (k + 1) * MW])
                return wc

            # x chunks 0,1 lead the two x queues so the first matmul's
            # moving operand lands as early as possible; W blocks 0 and 1
            # follow (split so the first converts start after one eighth of
            # a block).
            def x_chunk(k, queue):
                # Two half-DMAs (n0 then n1): the k-batch's n=0 matmuls and
                # the warm-up gate on half the bytes, halving arrival latency.
                th = x_pool.tile([P, T], F32R, tag=f"x_{k}", name=f"x_{k}")
                queue.dma_start(th[:, 0:TB], x_d[k * P : (k + 1) * P, 0:TB])
                queue.dma_start(th[:, TB:T], x_d[k * P : (k + 1) * P, TB:T])
                return th

            # First W quarter of blocks 0/1 leads each queue (128 KB each)
            # so the ScalarE convert stream starts ~3.5 us earlier and keeps
            # a cushion over the matmuls' weight consumption; x0/x1 follow
            # immediately (+0.4 us).
            ws0 = ws_pool.tile([P, KT * MW], FP8, tag="ws", name="ws_0")
            ws1 = ws_pool.tile([P, KT * MW], FP8, tag="ws", name="ws_1")
            QS = KT * MW // 8
            def w01_quarters(ws, wd_mb, queue, lo, hi):
                for i in range(lo, hi):
                    queue.dma_start(ws[:, i * QS : (i + 1) * QS],
                                    w_d[wd_mb, :, i * QS : (i + 1) * QS])
            w01_quarters(ws0, 0, nc.sync, 0, 1)
            w01_quarters(ws1, 1, nc.gpsimd, 0, 1)

            xt = [None] * KT
            xt[0] = x_chunk(0, nc.sync)
            xt[1] = x_chunk(1, nc.gpsimd)

            # PE warm-up: 30 matmuls reading x chunk 0 into a scratch PSUM
            # bank (results discarded). They run as soon as x0's first half
            # lands, while the rest of the inputs stream in, so the HAM
            # clock-gate is at 8/8 (2.4 GHz) when the first real matmul
            # issues — sized so the real stream starts just as the x supply
            # has enough lead to never gap >3.4 us (which would re-throttle
            # the PE to 1.2 GHz for ~10 us).
            warm_ps = psum_pool.tile([P, TB], F32, tag="psum", name="warm_ps")
            for i in range(30):
                nc.tensor.matmul(
                    warm_ps[:], xt[0][:, 0:P], xt[0][:, 0:TB],
                    start=True, stop=True, skip_group_check=True,
                )

            # Remaining W quarters of blocks 0/1 interleave with the early
            # x chunks: the super-block needs x_k every ~1.8 us but W
            # quarters only ahead of the convert stream.
            w01_quarters(ws0, 0, nc.sync, 1, 2)
            w01_quarters(ws1, 1, nc.gpsimd, 1, 2)
            xt[2] = x_chunk(2, nc.sync)
            xt[3] = x_chunk(3, nc.gpsimd)
            w01_quarters(ws0, 0, nc.sync, 2, 4)
            w01_quarters(ws1, 1, nc.gpsimd, 2, 4)
            xt[4] = x_chunk(4, nc.sync)
            xt[5] = x_chunk(5, nc.gpsimd)
            w01_quarters(ws0, 0, nc.sync, 4, 8)
            w01_quarters(ws1, 1, nc.gpsimd, 4, 8)

            # Interleaved converts (0,k),(1,k) for the fused first block.
            wcs01 = [[], []]
            for k in range(KT):
                wcs01[0].append(convert_w_chunk(0, ws0, k))
                wcs01[1].append(convert_w_chunk(1, ws1, k))

            # Remaining x: even chunks on sync, odd on gpsimd (behind W0/W1).
            # W blocks 2.. are staged via cheap triggers on the scalar
            # queue (emitted after the leading converts) so the two x
            # streams are never displaced; outputs ride sync only after the
            # x stream has fully landed, bias rides gpsimd likewise.
            for k in range(6, KT):
                xt[k] = x_chunk(k, nc.sync if k % 2 == 0 else nc.gpsimd)
            wstage_cache = {
                2: stage_w_block(2, nc.scalar),
                3: stage_w_block(3, nc.scalar),
            }
            bias_sb = b_pool.tile([P, MT], F32, tag="bias")
            nc.gpsimd.dma_start(bias_sb[:], b_d.rearrange("(mo p) -> p mo", p=P))

            nsls = [slice(n * TB, (n + 1) * TB) for n in range(NB)]

            def epilogue(m, n, ps):
                osb = out_pool.tile([P, TB], FP16, tag="osb",
                                    name=f"osb_{m}_{n}")
                nc.scalar.activation(
                    osb[:], ps[:], SIGN, bias=bias_sb[:, m : m + 1]
                )
                nc.sync.dma_start(out_d[m * P : (m + 1) * P, nsls[n]], osb[:])

            # ---- Fused first super-block: W blocks 0+1, 512 outf, all 8
            # PSUM banks. Its ~58 us of matmuls cover the x stream arrival.
            psums0 = {
                (mi, n): psum_pool.tile([P, TB], F32, tag="psum",
                                        name=f"ps0_{n}_{mi}")
                for mi in range(2 * M2)
                for n in range(NB)
            }

            wc_cache = {}
            KS = KT - 4  # tail k-steps run group-major so groups drain early
            for k in range(KS):
                for mi in range(2 * M2):
                    b, half = divmod(mi, 2)
                    msl = slice(half * P, (half + 1) * P)
                    for n in range(NB):
                        nc.tensor.matmul(
                            psums0[(mi, n)][:],
                            wcs01[b][k][:, msl],
                            xt[k][:, nsls[n]],
                            start=(k == 0),
                            stop=False,
                        )
            ws2 = wstage_cache.pop(2)
            wc_cache[2] = [convert_w_chunk_dve(2, ws2, k) for k in range(KT)]
            # Tail: finish each PSUM group separately and drain it at once,
            # so block 2's bank anti-dependencies clear before it needs them.
            for mi in range(2 * M2):
                b, half = divmod(mi, 2)
                msl = slice(half * P, (half + 1) * P)
                for n in range(NB):
                    for k in range(KS, KT):
                        nc.tensor.matmul(
                            psums0[(mi, n)][:],
                            wcs01[b][k][:, msl],
                            xt[k][:, nsls[n]],
                            start=False,
                            stop=(k == KT - 1),
                        )
                    epilogue(mi, n, psums0[(mi, n)])

            # ---- Steady-state blocks 2..15.
            for mb in range(2, MB):
                wcs = wc_cache.pop(mb)
                pf = mb + 2
                if pf < MB:
                    wstage_cache[pf] = stage_w_block(pf, nc.scalar)
                psums = {
                    (mi, n): psum_pool.tile([P, TB], F32, tag="psum",
                                            name=f"ps_{mb}_{n}_{mi}")
                    for mi in range(M2)
                    for n in range(NB)
                }
                last = mb == MB - 1
                if not last:
                    for k in range(KT):
                        for mi in range(M2):
                            msl = slice(mi * P, (mi + 1) * P)
                            for n in range(NB):
                                nc.tensor.matmul(
                                    psums[(mi, n)][:],
                                    wcs[k][:, msl],
                                    xt[k][:, nsls[n]],
                                    start=(k == 0),
                                    stop=(k == KT - 1),
                                )
                    if mb + 1 < MB:
                        wsn = wstage_cache.pop(mb + 1)
                        wc_cache[mb + 1] = [
                            convert_w_chunk_dve(mb + 1, wsn, k)
                            for k in range(KT)
                        ]
                    for mi in range(M2):
                        for n in range(NB):
                            epilogue(mb * M2 + mi, n, psums[(mi, n)])
                else:
                    # Last block group-major: each PSUM group finishes early
                    # so its epilogue overlaps the remaining matmuls.
                    for mi in range(M2):
                        msl = slice(mi * P, (mi + 1) * P)
                        for n in range(NB):
                            for k in range(KT):
                                nc.tensor.matmul(
                                    psums[(mi, n)][:],
                                    wcs[k][:, msl],
                                    xt[k][:, nsls[n]],
                                    start=(k == 0),
                                    stop=(k == KT - 1),
                                )
                            epilogue(mb * M2 + mi, n, psums[(mi, n)])
    nc.compile()
    _nc_cache = nc
    return nc


def prep_in_maps(x, weight, bias):
    """Host-side layout prep: transpose x, e4m3 sign(W) blocks, token shards."""
    x = np.asarray(x, dtype=np.float32)
    weight = np.asarray(weight, dtype=np.float32)
    bias = np.asarray(bias, dtype=np.float32)

    x_t = np.ascontiguousarray(x.T)  # [D_IN, N_TOK] fp32

    # sign(W).T [D_IN, D_OUT] -> block-major [MB][P, KT*256]:
    # w_blk8[mb, p, k*256+j] = sign(W).T[k*128+p, mb*256+j]
    s_t = np.sign(weight).T.astype(E4M3)  # [D_IN, D_OUT]
    w_blk8 = np.ascontiguousarray(
        s_t.reshape(KT, P, MB, MW)  # [k, p, mb, j]
        .transpose(2, 1, 0, 3)      # [mb, p, k, j]
        .reshape(MB, P, KT * MW)
    )

    in_maps = []
    for c in range(N_CORES):
        sl = slice(c * T, (c + 1) * T)
        in_maps.append(
            {
                "x_t": np.ascontiguousarray(x_t[:, sl]),
                "w_blk8": w_blk8,
                "bias": bias,
            }
        )
    return in_maps


def run(x, weight, bias, **spmd_kwargs):
    """Run on the 8 cores; returns (full_output, BassKernelResults)."""
    nc = build()
    in_maps = prep_in_maps(x, weight, bias)
    res = run_bass_kernel_spmd(nc, in_maps, core_ids=list(range(N_CORES)), **spmd_kwargs)
    out = np.empty((N_TOK, D_OUT), dtype=np.float32)
    for c in range(N_CORES):
        out[c * T : (c + 1) * T, :] = res.results[c]["out_t"].T.astype(np.float32)
    return out, res


def kernel(x, weight, bias):
    out, _ = run(x, weight, bias)
    return out
